# revision 1
# baseline (speedup 1.0000x reference)
"""Trainium2 Bass kernel for nn_DataTermLayer (data-term update of optical-flow).

Key observation: the reference's bilinear warp feeds *normalized* coords in
[-1, 1] straight into a pixel-space sampler, so after clipping the gather
only ever touches I1[b, 0:3, 0:3]. The whole layer therefore reduces to
elementwise math plus 9 per-image scalars:

  t2x = u + 2*w ; t2y = v + 2*h          (pre-division coords, f32-exact)
  nx  = t2x/511 - 1 ; ny = t2y/511 - 1   (device uses mult by r511)
  warped = [nx>=0 and ny>=0] * bilinear(P3x3, nx, ny)
  out_u = u - 0.1*(warped - I2)*(I1[h+1,w]-I1[h,w])
  out_v = v - 0.1*(warped - I2)*(I1[h,w+1]-I1[h,w])

Region split: warped is EXACTLY zero wherever 2w + u < 511 or 2h + v < 511
(the mask compare is done in pre-division space, bit-exact with the
reference's floor/clip branch decisions). With max|flow| ~ 5.4 that is all
cols < WZ~253 and all rows < HZ~253 -- ~74% of pixels only need
out = flow + 0.1*I2*grad. The full warp runs on the bottom-right quadrant
(col-sliced ops) and on a small 3-row "band" strip (rows HZ..255 of all
images gathered into one 16-partition tile).

The bilinear blend uses hat-basis coords mxp=min(t2*r511, 2),
e=relu(t2*r511-2) and per-image linear coefficients folded with the -0.1
scale on the host.

Sharding: pure data-parallel, 4 images per core across 8 cores.
"""
import sys

sys.path.insert(0, "/opt/trn_rl_repo")

import numpy as np

import concourse.bass as bass
import concourse.mybir as mybir
from concourse.bass_utils import run_bass_kernel_spmd
from concourse.tile import TileContext

F32 = mybir.dt.float32
ALU = mybir.AluOpType
ACTF = mybir.ActivationFunctionType

R511 = np.float32(1.0) / np.float32(511.0)
C1 = 511.0  # min f32 t with fl(t/511) >= 1  (verified exhaustively)


def build_nc(n_imgs: int = 4, n_rb: int = 4, wz: int = 253, hz: int = 253,
             legalize: bool = True):
    """One NeuronCore program: n_imgs images of [n_rb*128, 512].

    wz: first column where the warp can be nonzero (cols < wz are
    certainly zero-warp).  hz: same for rows.  Full-warp math runs on
    cols [wz, 512) of the bottom half plus a band strip rows [hz, 256).
    """
    assert n_rb == 4 and 225 <= hz <= 256 and 0 < wz <= 256
    W = 512
    H = n_rb * 128
    NBC = 256 - hz  # band compute rows per image (rows hz..255)
    NBR = NBC + 1   # band rows loaded per image (+1 for the row-shift grad)
    WF = W - wz     # full-math columns
    nc = bass.Bass()

    I1 = nc.dram_tensor("I1", [n_imgs, H + 1, W], F32, kind="ExternalInput")
    I2 = nc.dram_tensor("I2", [n_imgs, H, W], F32, kind="ExternalInput")
    FL = nc.dram_tensor("FL", [n_imgs, H, W, 2], F32, kind="ExternalInput")
    NCC = 9 * n_imgs + n_rb + 10  # +1 band gy2, +9 band-layout consts
    CC = nc.dram_tensor("CC", [128, NCC], F32, kind="ExternalInput")
    GX = nc.dram_tensor("GX", [128, 1024], F32, kind="ExternalInput")
    OUT = nc.dram_tensor("OUT", [n_imgs, H, W, 2], F32, kind="ExternalOutput")

    FDH = 1024  # free-dim of one half tile (2 row-blocks x 512)
    NBP = max(1, NBR * n_imgs)  # band partitions

    with TileContext(nc) as tc:
        with (
            tc.tile_pool(name="stat", bufs=1) as pstat,
            tc.tile_pool(name="pin", bufs=3) as pin,
            tc.tile_pool(name="ptmp", bufs=2) as ptmp,
            tc.tile_pool(name="pband", bufs=1) as pband,
        ):
            gx2 = pstat.tile([128, 1024], F32)
            nc.sync.dma_start(gx2[:], GX[:])
            cc = pstat.tile([128, NCC], F32)
            nc.sync.dma_start(cc[:], CC[:])
            cm2 = pstat.tile([128, 1], F32)
            nc.gpsimd.memset(cm2[:], -2.0)

            def cC(j):  # [128,1] column of cc
                return cc[:, j : j + 1]

            # ---------------- warp math on a generic tile set ---------------
            def warp_chain(pool, tag, P, fdims, t2x, t2y, i2v, bimg, dt_out):
                """Emit the warp pipeline writing 0.1*I2 + (-0.1)*zm*wr into
                dt_out.  t2x/t2y/i2v: APs with P partitions, fd free elems.
                bimg: image index for consts (None => band layout)."""
                cof = 9 * n_imgs + n_rb + 1

                def col(k, b):
                    c = cC(cof + k) if b is None else cC(9 * b + k)
                    return c[:P]

                shp = [P] + list(fdims)
                mxp = pool.tile(shp, F32, tag=f"{tag}mxp", name=f"{tag}mxp",
                                bufs=1)
                nc.vector.tensor_scalar(
                    mxp[:], t2x, float(R511), 2.0, ALU.mult, ALU.min
                )
                ex = pool.tile(shp, F32, tag=f"{tag}ex", name=f"{tag}ex", bufs=1)
                nc.scalar.activation(
                    ex[:], t2x, ACTF.Relu, bias=cm2[:P], scale=float(R511)
                )
                myp = pool.tile(shp, F32, tag=f"{tag}myp", name=f"{tag}myp", bufs=1)
                nc.vector.tensor_scalar(
                    myp[:], t2y, float(R511), 2.0, ALU.mult, ALU.min
                )
                ey = pool.tile(shp, F32, tag=f"{tag}ey", name=f"{tag}ey", bufs=1)
                nc.scalar.activation(
                    ey[:], t2y, ACTF.Relu, bias=cm2[:P], scale=float(R511)
                )
                lt = []
                for K in range(3):
                    # lt_K = mxp*beta'+alphat' and eg_K = ex*gamma' on ACT
                    # (interleaved so the DVE adds start early), add on DVE
                    ltK = pool.tile(shp, F32, tag=f"{tag}lt{K}",
                                    name=f"{tag}lt{K}", bufs=1)
                    nc.scalar.activation(
                        ltK[:], mxp[:], ACTF.Identity,
                        bias=col(3 * K + 1, bimg), scale=col(3 * K + 0, bimg),
                    )
                    eg = pool.tile(shp, F32, tag=f"{tag}eg",
                                   name=f"{tag}eg{K}", bufs=1)
                    nc.scalar.activation(
                        eg[:], ex[:], ACTF.Identity, bias=0.0,
                        scale=col(3 * K + 2, bimg),
                    )
                    nc.vector.tensor_tensor(ltK[:], eg[:], ltK[:], ALU.add)
                    lt.append(ltK)
                # wr = lin0 + (myp-1)*lin1 + ey*lin2 (into lt0), all on DVE to
                # avoid DVE->Pool->DVE ping-pong in the chain tail
                nc.vector.scalar_tensor_tensor(
                    lt[1][:], myp[:], 1.0, lt[1][:], ALU.subtract, ALU.mult
                )
                nc.vector.tensor_tensor(lt[2][:], ey[:], lt[2][:], ALU.mult)
                nc.vector.tensor_tensor(lt[0][:], lt[0][:], lt[1][:], ALU.add)
                nc.vector.tensor_tensor(lt[0][:], lt[0][:], lt[2][:], ALU.add)
                # masks + data term
                nc.vector.scalar_tensor_tensor(
                    lt[0][:], t2x, C1, lt[0][:], ALU.is_ge, ALU.mult
                )
                nc.vector.scalar_tensor_tensor(
                    lt[0][:], t2y, C1, lt[0][:], ALU.is_ge, ALU.mult
                )
                nc.vector.scalar_tensor_tensor(
                    dt_out, i2v, 0.1, lt[0][:], ALU.mult, ALU.add
                )

            # ---------------- band strip (rows hz..255, all imgs) -----------
            if NBC > 0:
                bi1 = pband.tile([NBP, 512], F32)
                bi1r = pband.tile([NBP, 512], F32)
                bi2 = pband.tile([NBP, 512], F32)
                bfl = pband.tile([NBP, 512, 2], F32)
                for b in range(n_imgs):
                    bsl = slice(NBR * b, NBR * (b + 1))
                    nc.sync.dma_start(bi1[bsl, :], I1[b, hz : hz + NBR, :])
                    nc.sync.dma_start(
                        bi1r[bsl, :], I1[b, hz + 1 : hz + 1 + NBR, :]
                    )
                    nc.sync.dma_start(bi2[bsl, :], I2[b, hz : hz + NBR, :])
                    nc.sync.dma_start(
                        bfl[bsl, :, :], FL[b, hz : hz + NBR, :, :]
                    )
                bu = bfl[:, :, 0]
                bv = bfl[:, :, 1]
                bt2x = pband.tile([NBP, 512], F32)
                nc.vector.tensor_tensor(bt2x[:], bu, gx2[:NBP, 0:512], ALU.add)
                bt2y = pband.tile([NBP, 512], F32)
                nc.scalar.activation(
                    bt2y[:], bv, ACTF.Identity,
                    bias=cC(9 * n_imgs + n_rb)[:NBP], scale=1.0,
                )
                bdt = pband.tile([NBP, 512], F32)
                warp_chain(pband, "bnd", NBP, [512], bt2x[:], bt2y[:],
                           bi2[:], None, bdt[:])
                bg1 = pband.tile([NBP, 512], F32)
                nc.vector.tensor_tensor(bg1[:], bi1r[:], bi1[:], ALU.subtract)
                bg2 = pband.tile([NBP, 512], F32)
                nc.vector.tensor_tensor(
                    bg2[:, 0:511], bi1[:, 1:512], bi1[:, 0:511], ALU.subtract
                )
                nc.gpsimd.memset(bg2[:, 511:512], 0.0)
                bmu = pband.tile([NBP, 512], F32)
                nc.gpsimd.tensor_tensor(bmu[:], bdt[:], bg1[:], ALU.mult)
                nc.vector.tensor_tensor(bu, bu, bmu[:], ALU.add)
                nc.gpsimd.tensor_tensor(bg2[:], bdt[:], bg2[:], ALU.mult)
                nc.vector.tensor_tensor(bv, bv, bg2[:], ALU.add)

            # ---------------- per image ------------------------------------
            for b in range(n_imgs):
                i1 = pin.tile([128, n_rb * 512], F32, tag="i1")
                nc.sync.dma_start(
                    i1[:].rearrange("p (rb w) -> p rb w", rb=n_rb),
                    I1[b, 0:H, :].rearrange("(rb p) w -> p rb w", p=128),
                )
                i1r = pin.tile([128, n_rb * 512], F32, tag="i1r")
                nc.sync.dma_start(
                    i1r[:].rearrange("p (rb w) -> p rb w", rb=n_rb),
                    I1[b, 1 : H + 1, :].rearrange("(rb p) w -> p rb w", p=128),
                )
                i2 = pin.tile([128, n_rb * 512], F32, tag="i2")
                nc.sync.dma_start(
                    i2[:].rearrange("p (rb w) -> p rb w", rb=n_rb),
                    I2[b].rearrange("(rb p) w -> p rb w", p=128),
                )
                fl = pin.tile([128, n_rb * 512, 2], F32, tag="fl")
                nc.sync.dma_start(
                    fl[:].rearrange("p (rb w) c -> p rb w c", rb=n_rb),
                    FL[b].rearrange("(rb p) w c -> p rb w c", p=128),
                )

                for hi in range(2):
                    hs = hi * FDH
                    hsl = slice(hs, hs + FDH)
                    u = fl[:, hsl, 0]
                    v = fl[:, hsl, 1]
                    i1h = i1[:, hsl]
                    i1rh = i1r[:, hsl]
                    i2h = i2[:, hsl]

                    g2 = ptmp.tile([128, FDH], F32, tag="g2", bufs=2)
                    nc.vector.tensor_tensor(
                        g2[:, 0:1023],
                        i1[:, hs + 1 : hs + 1024],
                        i1[:, hs : hs + 1023],
                        ALU.subtract,
                    )
                    g1 = ptmp.tile([128, FDH], F32, tag="g1", bufs=2)
                    nc.vector.tensor_tensor(g1[:], i1rh, i1h, ALU.subtract)
                    nc.gpsimd.memset(g2[:, 511:1024:512], 0.0)

                    dt = ptmp.tile([128, FDH], F32, tag="dt", bufs=2)
                    if hi == 0:
                        # top half: warp certainly zero -> dt = 0.1*I2
                        nc.vector.tensor_scalar_mul(dt[:], i2h, 0.1)
                    else:
                        # zero-warp columns
                        i2r = i2h.rearrange("p (r w) -> p r w", r=2)
                        dtr = dt[:].rearrange("p (r w) -> p r w", r=2)
                        nc.vector.tensor_scalar_mul(
                            dtr[:, :, 0:wz], i2r[:, :, 0:wz], 0.1
                        )
                        # full-math columns
                        ur = u.rearrange("p (r w) -> p r w", r=2)[:, :, wz:]
                        vr = v.rearrange("p (r w) -> p r w", r=2)[:, :, wz:]
                        i2f = i2r[:, :, wz:]
                        gxf = gx2[:].rearrange("p (r w) -> p r w", r=2)[
                            :, :, wz:
                        ]
                        t2x = ptmp.tile([128, 2, WF], F32, tag="t2x", bufs=1)
                        nc.vector.tensor_tensor(t2x[:], ur, gxf, ALU.add)
                        t2y = ptmp.tile([128, 2, WF], F32, tag="t2y", bufs=1)
                        for rbl in range(2):
                            nc.scalar.activation(
                                t2y[:, rbl, :], vr[:, rbl, :], ACTF.Identity,
                                bias=cC(9 * n_imgs + 2 + rbl), scale=1.0,
                            )
                        dtf = dtr[:, :, wz:]
                        warp_chain(ptmp, "f", 128, [2, WF], t2x[:], t2y[:],
                                   i2f, b, dtf)

                    # flow update (in place into fl tile)
                    nc.gpsimd.tensor_tensor(g1[:], dt[:], g1[:], ALU.mult)
                    nc.vector.tensor_tensor(u, u, g1[:], ALU.add)
                    nc.gpsimd.tensor_tensor(g2[:], dt[:], g2[:], ALU.mult)
                    nc.vector.tensor_tensor(v, v, g2[:], ALU.add)

                # patch band rows (overwrites the zero-branch values there)
                if NBC > 0:
                    nc.sync.dma_start(
                        fl[hz - 128 : hz - 128 + NBC, 512:1024, :],
                        bfl[NBR * b : NBR * b + NBC, :, :],
                    )

                for hi in range(2):
                    nc.sync.dma_start(
                        OUT[b, hi * 256 : hi * 256 + 256].rearrange(
                            "(rb p) w c -> p rb w c", p=128
                        ),
                        fl[:, hi * 1024 : hi * 1024 + 1024, :].rearrange(
                            "p (rb w) c -> p rb w c", rb=2
                        ),
                    )
    if legalize:
        legalize_single_wait(nc)
    return nc


# ---------------------------------------------------------------------------
# Post-pass: this walrus build encodes a single sync-wait slot per TPB
# instruction. Tile's sem assignment can emit 2+ waits on one instruction;
# hoist all but the last wait onto same-engine EventSemaphore carriers placed
# immediately before it (the sequencer then waits sequentially, which is
# semantically identical).
def legalize_single_wait(nc):
    import bass_rust

    capped = {
        mybir.EngineType.Activation,
        mybir.EngineType.DVE,
        mybir.EngineType.Pool,
        mybir.EngineType.PE,
        mybir.EngineType.SP,
    }
    exempt = {"EventSemaphore", "NoOp", "TriggerDma"}
    n = 0
    for fn in nc.m.functions:
        for blk in fn.blocks:
            insts = blk.instructions  # live list
            rebuilt = []
            changed = False
            for inst in list(insts):
                si = inst.sync_info
                waits = list(si.on_wait) if si is not None else []
                if (
                    len(waits) > 1
                    and inst.engine in capped
                    and str(inst.opcode) not in exempt
                ):
                    for w in waits[:-1]:
                        ev = mybir.InstEventSemaphore(
                            name=f"waitcarrier_{inst.name}_{n}", ins=[], outs=[]
                        )
                        ev.engine = inst.engine
                        ev.sync_info = bass_rust.SyncInfo(
                            on_wait=[w], on_update=[]
                        )
                        rebuilt.append(ev)
                        n += 1
                    inst.sync_info = bass_rust.SyncInfo(
                        on_wait=[waits[-1]], on_update=list(si.on_update)
                    )
                    changed = True
                rebuilt.append(inst)
            if changed:
                insts[:] = rebuilt
    return n


def host_consts(I1c: np.ndarray, n_rb: int = 4, hz: int = 253) -> np.ndarray:
    """Per-image folded warp coefficients + per-partition 2*h columns.

    I1c: [n_imgs, H, W] float32.  Returns [128, 9*n_imgs + n_rb + 10] f32.
    Per image b, cols 9*b+3*K+(0:beta', 1:alphat', 2:gamma').
    Col 9n+rb: 2*(128*rb+p).  Col 9n+n_rb: band 2*h.  Cols 9n+n_rb+1..+9:
    band-partition-layout consts (partition 4b+r holds image b's values).
    """
    f = np.float32
    n_imgs = I1c.shape[0]
    cc = np.zeros((128, 9 * n_imgs + n_rb + 10), dtype=np.float32)
    m01 = f(-0.1)
    allc = np.zeros((n_imgs, 9), dtype=np.float32)
    for b in range(n_imgs):
        P = I1c[b, 0:3, 0:3].astype(np.float32)
        d1 = (P[:, 1] - P[:, 0]).astype(f)
        d2 = (P[:, 2] - P[:, 1]).astype(f)
        alpha = np.array(
            [P[0, 0], f(P[1, 0] - P[0, 0]), f(P[2, 0] - P[1, 0])], dtype=f
        )
        beta = np.array([d1[0], f(d1[1] - d1[0]), f(d1[2] - d1[1])], dtype=f)
        gamma = np.array([d2[0], f(d2[1] - d2[0]), f(d2[2] - d2[1])], dtype=f)
        for K in range(3):
            allc[b, 3 * K + 0] = f(m01 * beta[K])
            allc[b, 3 * K + 1] = f(m01 * f(alpha[K] - beta[K]))
            allc[b, 3 * K + 2] = f(m01 * gamma[K])
        cc[:, 9 * b : 9 * b + 9] = allc[b][None, :]
    p = np.arange(128, dtype=np.float32)
    for rb in range(n_rb):
        cc[:, 9 * n_imgs + rb] = f(2.0) * (f(128.0 * rb) + p)
    # band columns (NBR = 257-hz rows per image)
    base = 9 * n_imgs + n_rb
    nbr = 257 - hz
    for b in range(n_imgs):
        for r in range(nbr):
            pp = nbr * b + r
            if pp < 128:
                cc[pp, base] = f(2.0) * f(hz + r)
                cc[pp, base + 1 : base + 10] = allc[b]
    return cc


def host_gx() -> np.ndarray:
    w2 = (np.float32(2.0) * np.arange(512, dtype=np.float32)).astype(np.float32)
    return np.tile(w2, (128, 2)).astype(np.float32)


_NC = None
_NC_KEY = None


def _get_nc(wz, hz):
    global _NC, _NC_KEY
    if _NC is None or _NC_KEY != (wz, hz):
        _NC = build_nc(4, 4, wz=wz, hz=hz)
        _NC_KEY = (wz, hz)
    return _NC


def _splits(flow):
    umax = float(max(flow[..., 0].max(), 0.0))
    vmax = float(max(flow[..., 1].max(), 0.0))
    # first col/row where 2*x + d can reach 511.0 (f32-exact threshold)
    wz = int(min(256, max(1, (511.0 - umax) // 2 + 1)))
    hz = int(min(256, max(225, (511.0 - vmax) // 2 + 1)))
    # paranoia: verify in f32 exactly like the device compare
    assert np.float32(2.0 * (wz - 1)) + np.float32(umax) < np.float32(511.0)
    assert np.float32(2.0 * (hz - 1)) + np.float32(vmax) < np.float32(511.0)
    return wz, hz


def _make_in_maps(I1, I2, flow, wz, hz, n_cores=8):
    per = I1.shape[0] // n_cores
    gx = host_gx()
    in_maps = []
    for c in range(n_cores):
        sl = slice(c * per, (c + 1) * per)
        i1c = np.ascontiguousarray(I1[sl, :, :, 0], dtype=np.float32)
        i1pad = np.concatenate([i1c, i1c[:, -1:, :]], axis=1)
        in_maps.append(
            {
                "I1": np.ascontiguousarray(i1pad),
                "I2": np.ascontiguousarray(I2[sl, :, :, 0], dtype=np.float32),
                "FL": np.ascontiguousarray(flow[sl], dtype=np.float32),
                "CC": host_consts(i1c, 4, hz),
                "GX": gx,
            }
        )
    return in_maps


def run(I1, I2, flow, trace=False, **kw):
    wz, hz = _splits(np.asarray(flow))
    nc = _get_nc(wz, hz)
    in_maps = _make_in_maps(I1, I2, flow, wz, hz)
    res = run_bass_kernel_spmd(nc, in_maps, list(range(8)), trace=trace, **kw)
    out = np.concatenate([r["OUT"] for r in res.results], axis=0)
    return out, res


def kernel(I1, I2, flow):
    out, _ = run(I1, I2, flow)
    return out.astype(np.float32)



# revision 6
# speedup vs baseline: 1.1172x; 1.1172x over previous
"""Trainium2 Bass kernel for nn_DataTermLayer (data-term update of optical-flow).

Key observation: the reference's bilinear warp feeds *normalized* coords in
[-1, 1] straight into a pixel-space sampler, so after clipping the gather
only ever touches I1[b, 0:3, 0:3]. The whole layer therefore reduces to
elementwise math plus 9 per-image scalars:

  t2x = u + 2*w ; t2y = v + 2*h          (pre-division coords, f32-exact)
  x   = t2x/511 - 1 ; y = t2y/511 - 1
  warped = [x>=0][y>=0] * bilinear3x3(P, x, y)
  out_u = u - 0.1*(warped - I2)*(I1[h+1,w]-I1[h,w])
  out_v = v - 0.1*(warped - I2)*(I1[h,w+1]-I1[h,w])

This version (vs. the 157us baseline):
  * The row gradient G1 = 0.1*(I1[h+1,w]-I1[h,w]) is computed on the idle
    PE via a float32r shift-matmul (lhsT = +-0.1 bidiagonal), which kills
    both the duplicate shifted-I1 HBM load (-4MB/core) and the DVE
    subtract pass.  u_next = u + (I2 - warped)*G1 with G1 read from PSUM.
  * warped is expanded EXACTLY in the basis (1,t2x,EX)x(1,t2y,EY) with
    EX=relu(t2x-1022), EY=relu(t2y-1022) and 9 per-image consts F[i,j]
    (folded with -0.1 on the host): 8 ACT + 9 DVE-class passes on the
    bottom-right warp quadrant only.
  * dt0 = 0.1*I2 runs on the ACT engine; every TT-class pass is split
    along the free dim between DVE and Pool (fd = DVE share) so both
    stay busy; masks/updates unchanged from the verified baseline.
  * The zero-warp region logic and the 3-row "band" strip are unchanged
    in spirit from the baseline (warped == 0 wherever 2w+u < 511 or
    2h+v < 511, compared in pre-division space, bit-exact).

Sharding: pure data-parallel, 4 images per core across 8 cores.
"""
import sys

sys.path.insert(0, "/opt/trn_rl_repo")

import numpy as np

import concourse.bass as bass
import concourse.mybir as mybir
from concourse.bass_utils import run_bass_kernel_spmd
from concourse.tile import TileContext

F32 = mybir.dt.float32
F32R = mybir.dt.float32r
ALU = mybir.AluOpType
ACTF = mybir.ActivationFunctionType

C1 = 511.0  # min f32 t with fl(t/511) >= 1  (verified exhaustively)
FD = 0.61   # DVE share of every split TT-class pass (rest on Pool)


def build_nc(n_imgs: int = 4, n_rb: int = 4, wz: int = 253, hz: int = 253,
             legalize: bool = True, fd: float = FD):
    """One NeuronCore program: n_imgs images of [512, 512].

    wz: first column where the warp can be nonzero.  hz: same for rows.
    Warp math runs on cols [wz, 512) of rb-blocks 2,3 plus a small band
    strip rows [hz, 256) of all images gathered into one tile.
    """
    assert n_rb == 4 and 225 <= hz <= 256 and 0 < wz <= 256
    W = 512
    H = n_rb * 128
    NBC = 256 - hz  # band compute rows per image (rows hz..255)
    NBR = NBC + 1   # band rows loaded per image (+1 for the row-shift grad)
    WF = W - wz     # warp-math columns
    nc = bass.Bass()

    I1 = nc.dram_tensor("I1", [n_imgs, H, W], F32, kind="ExternalInput")
    I2 = nc.dram_tensor("I2", [n_imgs, H, W], F32, kind="ExternalInput")
    FL = nc.dram_tensor("FL", [n_imgs, H, W, 2], F32, kind="ExternalInput")
    NCC = 9 * n_imgs + n_rb + 10
    CC = nc.dram_tensor("CC", [128, NCC], F32, kind="ExternalInput")
    GX = nc.dram_tensor("GX", [128, 1024], F32, kind="ExternalInput")
    SM = nc.dram_tensor("SM", [128, 384], F32, kind="ExternalInput")
    OUT = nc.dram_tensor("OUT", [n_imgs, H, W, 2], F32, kind="ExternalOutput")

    NBP = max(1, NBR * n_imgs)  # band partitions

    with TileContext(nc) as tc:
        with (
            tc.tile_pool(name="stat", bufs=1) as pstat,
            tc.tile_pool(name="pin", bufs=2) as pin,
            tc.tile_pool(name="ptmp", bufs=2) as ptmp,
            tc.tile_pool(name="pwarp", bufs=2) as pwarp,
            tc.tile_pool(name="pband", bufs=1) as pband,
            tc.tile_pool(name="pps", bufs=2, space="PSUM") as pps,
        ):
            gx2 = pstat.tile([128, 1024], F32)
            nc.sync.dma_start(gx2[:], GX[:])
            cc = pstat.tile([128, NCC], F32)
            nc.sync.dma_start(cc[:], CC[:])
            sm = pstat.tile([128, 384], F32)
            nc.sync.dma_start(sm[:], SM[:])
            cmth = pstat.tile([128, 1], F32)
            nc.gpsimd.memset(cmth[:], -1022.0)

            def cC(j):  # [128,1] column of cc
                return cc[:, j : j + 1]

            # --------- split helpers: DVE gets fd of the last axis ---------
            def _sp(ap, lo, hi):
                if ap.shape == ():  # scalar marker never happens
                    return ap
                idx = [slice(None)] * (len(ap.shape) - 1) + [slice(lo, hi)]
                return ap[tuple(idx)]

            def tts(out, a, b_, op):
                n = out.shape[-1]
                k = max(1, min(n - 1, int(n * fd + 0.5))) if n > 1 else n
                nc.vector.tensor_tensor(_sp(out, 0, k), _sp(a, 0, k),
                                        _sp(b_, 0, k), op)
                if k < n:
                    nc.gpsimd.tensor_tensor(_sp(out, k, n), _sp(a, k, n),
                                            _sp(b_, k, n), op)

            def stts(out, a, scl, b_, op0, op1):
                # scalar_tensor_tensor only exists on DVE
                nc.vector.scalar_tensor_tensor(out, a, scl, b_, op0, op1)

            # ---------------- warp math on a generic tile set ---------------
            def warp_chain(pool, tag, P, fdims, t2x, t2y, bimg):
                """Emit wm = -0.1*warped*[t2x>=C1][t2y>=C1] into a fresh tile.

                Exact basis: warped = sum_ij F[i,j]*ay_i*ax_j with
                ax=(1,t2x,EX), ay=(1,t2y,EY), EX=relu(t2x-1022).
                bimg: image index for consts (None => band layout).
                """
                cof = 9 * n_imgs + n_rb + 1

                def col(k):
                    c = cC(cof + k) if bimg is None else cC(9 * bimg + k)
                    return c[:P]

                shp = [P] + list(fdims)

                def T(nm, bufs=1):
                    return pool.tile(shp, F32, tag=f"{tag}{nm}",
                                     name=f"{tag}{nm}", bufs=bufs)

                ex = T("ex")
                nc.scalar.activation(ex[:], t2x, ACTF.Relu, bias=cmth[:P],
                                     scale=1.0)
                ey = T("ey")
                nc.scalar.activation(ey[:], t2y, ACTF.Relu, bias=cmth[:P],
                                     scale=1.0)
                pqr = []
                for i in range(3):
                    ti = T(f"pqr{i}")
                    nc.scalar.activation(ti[:], t2x, ACTF.Identity,
                                         bias=col(3 * i + 0),
                                         scale=col(3 * i + 1))
                    eg = T("eg", bufs=2)
                    nc.scalar.activation(eg[:], ex[:], ACTF.Identity,
                                         bias=0.0, scale=col(3 * i + 2))
                    tts(ti[:], ti[:], eg[:], ALU.add)
                    pqr.append(ti)
                pt, qt, rt = pqr
                tts(qt[:], t2y, qt[:], ALU.mult)
                tts(rt[:], ey[:], rt[:], ALU.mult)
                tts(pt[:], pt[:], qt[:], ALU.add)
                tts(pt[:], pt[:], rt[:], ALU.add)
                stts(pt[:], t2x, C1, pt[:], ALU.is_ge, ALU.mult)
                stts(pt[:], t2y, C1, pt[:], ALU.is_ge, ALU.mult)
                return pt

            # ---------------- band strip (rows hz..255, all imgs) -----------
            if NBC > 0:
                bi1 = pband.tile([NBP, 512], F32)
                bi1r = pband.tile([NBP, 512], F32)
                bi2 = pband.tile([NBP, 512], F32)
                bfl = pband.tile([NBP, 512, 2], F32)
                for b in range(n_imgs):
                    bsl = slice(NBR * b, NBR * (b + 1))
                    nc.sync.dma_start(bi1[bsl, :], I1[b, hz : hz + NBR, :])
                    nc.sync.dma_start(
                        bi1r[bsl, :], I1[b, hz + 1 : hz + 1 + NBR, :]
                    )
                    nc.sync.dma_start(bi2[bsl, :], I2[b, hz : hz + NBR, :])
                    nc.sync.dma_start(
                        bfl[bsl, :, :], FL[b, hz : hz + NBR, :, :]
                    )
                bu = bfl[:, :, 0]
                bv = bfl[:, :, 1]
                bt2x = pband.tile([NBP, 512], F32)
                tts(bt2x[:], bu, gx2[:NBP, 0:512], ALU.add)
                bt2y = pband.tile([NBP, 512], F32)
                nc.scalar.activation(
                    bt2y[:], bv, ACTF.Identity,
                    bias=cC(9 * n_imgs + n_rb)[:NBP], scale=1.0,
                )
                wmB = warp_chain(pband, "bnd", NBP, [512], bt2x[:], bt2y[:],
                                 None)
                bdt = pband.tile([NBP, 512], F32)
                stts(bdt[:], bi2[:], 0.1, wmB[:], ALU.mult, ALU.add)
                bg1 = pband.tile([NBP, 512], F32)
                tts(bg1[:], bi1r[:], bi1[:], ALU.subtract)
                bg2 = pband.tile([NBP, 512], F32)
                nc.vector.tensor_tensor(
                    bg2[:, 0:511], bi1[:, 1:512], bi1[:, 0:511], ALU.subtract
                )
                nc.gpsimd.memset(bg2[:, 511:512], 0.0)
                tts(bg1[:], bdt[:], bg1[:], ALU.mult)
                tts(bu, bu, bg1[:], ALU.add)
                tts(bg2[:], bdt[:], bg2[:], ALU.mult)
                tts(bv, bv, bg2[:], ALU.add)

            # ---------------- per image ------------------------------------
            for b in range(n_imgs):
                i1 = pin.tile([128, n_rb * 512], F32, tag="i1")
                nc.sync.dma_start(
                    i1[:].rearrange("p (rb w) -> p rb w", rb=n_rb),
                    I1[b].rearrange("(rb p) w -> p rb w", p=128),
                )
                i2 = pin.tile([128, n_rb * 512], F32, tag="i2")
                nc.sync.dma_start(
                    i2[:].rearrange("p (rb w) -> p rb w", rb=n_rb),
                    I2[b].rearrange("(rb p) w -> p rb w", p=128),
                )
                fl = pin.tile([128, n_rb * 512, 2], F32, tag="fl")
                nc.sync.dma_start(
                    fl[:].rearrange("p (rb w) c -> p rb w c", rb=n_rb),
                    FL[b].rearrange("(rb p) w c -> p rb w c", p=128),
                )

                # PE: G1 = 0.1*(I1[r+1]-I1[r]) into PSUM, row 511 -> 0
                ps = pps.tile([128, n_rb * 512], F32, tag="ps")
                smS = sm[:, 0:128]
                smP = sm[:, 128:256]
                smL = sm[:, 256:384]
                for rb in range(n_rb):
                    dst = ps[:, rb * 512 : (rb + 1) * 512]
                    rhs = i1[:, rb * 512 : (rb + 1) * 512]
                    if rb < n_rb - 1:
                        nc.tensor.matmul(dst, smS, rhs, start=True, stop=False)
                        rhs2 = i1[:, (rb + 1) * 512 : (rb + 2) * 512]
                        nc.tensor.matmul(dst, smP, rhs2, start=False,
                                         stop=True)
                    else:
                        nc.tensor.matmul(dst, smL, rhs, start=True, stop=True)

                # dt0 = 0.1*I2 (ACT), g2 = col-gradient of I1
                dt0 = ptmp.tile([128, n_rb * 512], F32, tag="dt0")
                nc.scalar.activation(dt0[:], i2[:], ACTF.Identity, bias=0.0,
                                     scale=0.1)
                g2 = ptmp.tile([128, n_rb * 512], F32, tag="g2")
                NW = n_rb * 512
                tts(g2[:, 0 : NW - 1], i1[:, 1:NW], i1[:, 0 : NW - 1],
                    ALU.subtract)
                g2r = g2[:].rearrange("p (r w) -> p r w", r=n_rb)
                nc.gpsimd.memset(g2r[:, :, 511:512], 0.0)

                # ---- warp quadrant: rb 2,3  cols [wz,512) ----
                flv = fl[:].rearrange("p (r w) c -> p r w c", r=n_rb)
                ur = flv[:, 2:4, wz:, 0]
                vr = flv[:, 2:4, wz:, 1]
                i2v = i2[:].rearrange("p (r w) -> p r w", r=n_rb)[:, 2:4, wz:]
                psv = ps[:].rearrange("p (r w) -> p r w", r=n_rb)[:, 2:4, wz:]
                dt0v = dt0[:].rearrange("p (r w) -> p r w", r=n_rb)[
                    :, 2:4, wz:
                ]
                gxf = gx2[:].rearrange("p (r w) -> p r w", r=2)[:, :, wz:]

                t2x = pwarp.tile([128, 2, WF], F32, tag="t2x")
                tts(t2x[:], ur, gxf, ALU.add)
                t2y = pwarp.tile([128, 2, WF], F32, tag="t2y")
                for rbl in range(2):
                    nc.scalar.activation(
                        t2y[:, rbl, :], vr[:, rbl, :], ACTF.Identity,
                        bias=cC(9 * n_imgs + 2 + rbl), scale=1.0,
                    )
                wm = warp_chain(pwarp, "w", 128, [2, WF], t2x[:], t2y[:], b)
                # dtv (v-path): dt0 += wm on the quadrant
                tts(dt0v, dt0v, wm[:], ALU.add)
                # dtu (u-path): wtmp = 10*wm + I2  (= I2 - warped, masked)
                wtmp = pwarp.tile([128, 2, WF], F32, tag="wtmp")
                stts(wtmp[:], wm[:], 10.0, i2v, ALU.mult, ALU.add)

                # ---- full-width updates ----
                # m1 = I2 * G1   (in place onto i2; PSUM src => DVE only)
                nc.vector.tensor_tensor(i2[:], i2[:], ps[:], ALU.mult)
                # fix quadrant: m1 = (I2 - warped)*G1
                nc.vector.tensor_tensor(i2v, wtmp[:], psv, ALU.mult)
                flu = fl[:, :, 0]
                tts(flu, flu, i2[:], ALU.add)
                # m2 = dt0 * g2 (in place onto g2)
                tts(g2[:], dt0[:], g2[:], ALU.mult)
                flv2 = fl[:, :, 1]
                tts(flv2, flv2, g2[:], ALU.add)

                # patch band rows (overwrites the zero-branch values there)
                if NBC > 0:
                    nc.sync.dma_start(
                        flv[hz - 128 : hz - 128 + NBC, 1, :, :],
                        bfl[NBR * b : NBR * b + NBC, :, :],
                    )

                nc.sync.dma_start(
                    OUT[b].rearrange("(rb p) w c -> p rb w c", p=128),
                    fl[:].rearrange("p (rb w) c -> p rb w c", rb=n_rb),
                )
    if legalize:
        legalize_single_wait(nc)
    return nc


# ---------------------------------------------------------------------------
# Post-pass: this walrus build encodes a single sync-wait slot per TPB
# instruction. Tile's sem assignment can emit 2+ waits on one instruction;
# hoist all but the last wait onto same-engine EventSemaphore carriers placed
# immediately before it (the sequencer then waits sequentially, which is
# semantically identical).
def legalize_single_wait(nc):
    import bass_rust

    capped = {
        mybir.EngineType.Activation,
        mybir.EngineType.DVE,
        mybir.EngineType.Pool,
        mybir.EngineType.PE,
        mybir.EngineType.SP,
    }
    exempt = {"EventSemaphore", "NoOp", "TriggerDma"}
    n = 0
    for fn in nc.m.functions:
        for blk in fn.blocks:
            insts = blk.instructions  # live list
            rebuilt = []
            changed = False
            for inst in list(insts):
                si = inst.sync_info
                waits = list(si.on_wait) if si is not None else []
                if (
                    len(waits) > 1
                    and inst.engine in capped
                    and str(inst.opcode) not in exempt
                ):
                    for w in waits[:-1]:
                        ev = mybir.InstEventSemaphore(
                            name=f"waitcarrier_{inst.name}_{n}", ins=[], outs=[]
                        )
                        ev.engine = inst.engine
                        ev.sync_info = bass_rust.SyncInfo(
                            on_wait=[w], on_update=[]
                        )
                        rebuilt.append(ev)
                        n += 1
                    inst.sync_info = bass_rust.SyncInfo(
                        on_wait=[waits[-1]], on_update=list(si.on_update)
                    )
                    changed = True
                rebuilt.append(inst)
            if changed:
                insts[:] = rebuilt
    return n


def _img_consts(P3: np.ndarray) -> np.ndarray:
    """9 warp consts F[i,j] (row-major) for one image's 3x3 corner P3[y,x].

    warped = sum_ij F'[i,j]*ay_i*ax_j, ax=(1,t2x,relu(t2x-1022)),
    ay=(1,t2y,relu(t2y-1022));  F = -0.1*F'.
    """
    P = P3.astype(np.float64)
    E = np.stack([P[:, 0], P[:, 1] - P[:, 0], P[:, 2] - P[:, 1]], axis=1)
    D = np.stack([E[0], E[1] - E[0], E[2] - E[1]], axis=0)
    r = 1.0 / 511.0
    Mx = np.array([[1.0, 0.0, 0.0], [-1.0, r, -r], [0.0, 0.0, r]])
    F = -0.1 * (Mx.T @ D @ Mx)
    return F.reshape(-1).astype(np.float32)


def host_consts(I1c: np.ndarray, n_rb: int = 4, hz: int = 253) -> np.ndarray:
    """Per-image folded warp coefficients + per-partition 2*h columns.

    I1c: [n_imgs, H, W] float32.  Returns [128, 9*n_imgs + n_rb + 10] f32.
    Per image b, cols 9*b+3*i+j hold F[i,j].  Col 9n+rb: 2*(128*rb+p).
    Col 9n+n_rb: band 2*h.  Cols 9n+n_rb+1..+9: band-partition-layout
    consts (partition NBR*b+r holds image b's values).
    """
    f = np.float32
    n_imgs = I1c.shape[0]
    cc = np.zeros((128, 9 * n_imgs + n_rb + 10), dtype=np.float32)
    allc = np.zeros((n_imgs, 9), dtype=np.float32)
    for b in range(n_imgs):
        allc[b] = _img_consts(I1c[b, 0:3, 0:3])
        cc[:, 9 * b : 9 * b + 9] = allc[b][None, :]
    p = np.arange(128, dtype=np.float32)
    for rb in range(n_rb):
        cc[:, 9 * n_imgs + rb] = f(2.0) * (f(128.0 * rb) + p)
    # band columns (NBR = 257-hz rows per image)
    base = 9 * n_imgs + n_rb
    nbr = 257 - hz
    for b in range(n_imgs):
        for r in range(nbr):
            pp = nbr * b + r
            if pp < 128:
                cc[pp, base] = f(2.0) * f(hz + r)
                cc[pp, base + 1 : base + 10] = allc[b]
    return cc


def host_gx() -> np.ndarray:
    w2 = (np.float32(2.0) * np.arange(512, dtype=np.float32)).astype(np.float32)
    return np.tile(w2, (128, 2)).astype(np.float32)


def host_sm() -> np.ndarray:
    """[128, 384]: cols 0:128 = shift lhsT S (S[k,m]: +0.1 at k=m+1,
    -0.1 at k=m), cols 128:256 = patch lhsT (0.1 at k=0, m=127),
    cols 256:384 = S with column 127 zeroed (last row-block: dy row
    511 must be exactly 0)."""
    sm = np.zeros((128, 384), dtype=np.float32)
    a = np.float32(0.1)
    for m in range(128):
        sm[m, m] = -a
        if m + 1 < 128:
            sm[m + 1, m] = a
    sm[0, 128 + 127] = a
    sm[:, 256:384] = sm[:, 0:128]
    sm[127, 256 + 127] = 0.0
    return sm


_NC = None
_NC_KEY = None


def _get_nc(wz, hz):
    global _NC, _NC_KEY
    if _NC is None or _NC_KEY != (wz, hz):
        _NC = build_nc(4, 4, wz=wz, hz=hz)
        _NC_KEY = (wz, hz)
    return _NC


def _splits(flow):
    umax = float(max(flow[..., 0].max(), 0.0))
    vmax = float(max(flow[..., 1].max(), 0.0))
    # first col/row where 2*x + d can reach 511.0 (f32-exact threshold)
    wz = int(min(256, max(1, (511.0 - umax) // 2 + 1)))
    hz = int(min(256, max(225, (511.0 - vmax) // 2 + 1)))
    # paranoia: verify in f32 exactly like the device compare
    assert np.float32(2.0 * (wz - 1)) + np.float32(umax) < np.float32(511.0)
    assert np.float32(2.0 * (hz - 1)) + np.float32(vmax) < np.float32(511.0)
    return wz, hz


def _make_in_maps(I1, I2, flow, wz, hz, n_cores=8):
    per = I1.shape[0] // n_cores
    gx = host_gx()
    sm = host_sm()
    in_maps = []
    for c in range(n_cores):
        sl = slice(c * per, (c + 1) * per)
        i1c = np.ascontiguousarray(I1[sl, :, :, 0], dtype=np.float32)
        in_maps.append(
            {
                "I1": i1c,
                "I2": np.ascontiguousarray(I2[sl, :, :, 0], dtype=np.float32),
                "FL": np.ascontiguousarray(flow[sl], dtype=np.float32),
                "CC": host_consts(i1c, 4, hz),
                "GX": gx,
                "SM": sm,
            }
        )
    return in_maps


def run(I1, I2, flow, trace=False, **kw):
    wz, hz = _splits(np.asarray(flow))
    nc = _get_nc(wz, hz)
    in_maps = _make_in_maps(I1, I2, flow, wz, hz)
    res = run_bass_kernel_spmd(nc, in_maps, list(range(8)), trace=trace, **kw)
    out = np.concatenate([r["OUT"] for r in res.results], axis=0)
    return out, res


def kernel(I1, I2, flow):
    out, _ = run(I1, I2, flow)
    return out.astype(np.float32)


# revision 8
# speedup vs baseline: 1.2644x; 1.1318x over previous
"""Trainium2 Bass kernel for nn_DataTermLayer (data-term update of optical-flow).

Key observation: the reference's bilinear warp feeds *normalized* coords in
[-1, 1] straight into a pixel-space sampler, so after clipping the gather
only ever touches I1[b, 0:3, 0:3]. The whole layer reduces to elementwise
math plus 9 per-image scalars:

  t2x = u + 2*w ; t2y = v + 2*h          (pre-division coords, f32-exact)
  x   = t2x/511 - 1 ; y = t2y/511 - 1
  warped = [x>=0][y>=0] * bilinear3x3(P, x, y)
  dt    = 0.1*(I2 - warped)
  out_u = u + dt*(I1[h+1,w]-I1[h,w]) ; out_v = v + dt*(I1[h,w+1]-I1[h,w])

Structure (2e-2 rel tolerance; measured ~1e-4):
  * I1 is cast once to bf16; the row gradient comes from the idle PE as a
    +-1 bidiagonal bf16 shift-matmul into PSUM (kills the baseline's
    duplicate shifted-I1 HBM load and the DVE subtract), and the column
    gradient is a 2x-rate bf16 DVE subtract.
  * dt0 = 0.1*I2 (bf16, ACT engine).  warped is expanded EXACTLY in the
    basis (1,t2x,EX)x(1,t2y,EY), EX=relu(t2x-1022): on the bottom-right
    warp quadrant only the 4 EX/EY-free terms run full-size; the EX terms
    live only in the last ~3 columns and EY in the last ~3 rows, patched
    by tiny strip ops (the Y strip runs on partitions 96:128 where
    EY==0 rows self-cancel).  Masks are f32-exact compares vs 511 in
    pre-division space (warped == 0 wherever 2w+u < 511 or 2h+v < 511).
  * A 3-row "band" strip (rows hz..255 of all images in one tile) redoes
    the rows adjacent to the half boundary with the full chain, as in
    the baseline.
  * The flow updates run on the GpSimd engine, everything PSUM-touching
    on DVE, single-source ops on ACT; output DMAs trigger from the ACT
    queue so they never block the SP input-DMA stream.

Sharding: pure data-parallel, 4 images per core across 8 cores.
"""
import sys

sys.path.insert(0, "/opt/trn_rl_repo")

import numpy as np
import ml_dtypes

import concourse.bass as bass
import concourse.mybir as mybir
from concourse.bass_utils import run_bass_kernel_spmd
from concourse.tile import TileContext

F32 = mybir.dt.float32
BF16 = mybir.dt.bfloat16
ALU = mybir.AluOpType
ACTF = mybir.ActivationFunctionType

C1 = 511.0  # min f32 t with fl(t/511) >= 1  (verified exhaustively)


def build_nc(n_imgs: int = 4, n_rb: int = 4, wz: int = 253, hz: int = 253,
             wze: int = 509, hze: int = 509, legalize: bool = True):
    """One NeuronCore program: n_imgs images of [512, 512].

    wz/hz: first col/row where the warp can be nonzero (t2 >= 511
    reachable).  wze/hze: first col/row where EX/EY (t2 >= 1022) can be
    nonzero.
    """
    assert n_rb == 4 and 225 <= hz <= 256 and 0 < wz <= 256
    assert wz < wze <= 512 and max(hz, 480) < hze <= 512
    W = 512
    H = n_rb * 128
    NBC = 256 - hz  # band compute rows per image (rows hz..255)
    NBR = NBC + 1   # band rows loaded per image (+1 for the row-shift grad)
    WF = W - wz     # warp-math columns
    WE = W - wze    # EX strip columns
    XS = wze - wz   # EX strip offset inside the warp quadrant
    nc = bass.Bass()

    I1 = nc.dram_tensor("I1", [n_imgs, H, W], F32, kind="ExternalInput")
    I2 = nc.dram_tensor("I2", [n_imgs, H, W], F32, kind="ExternalInput")
    FL = nc.dram_tensor("FL", [n_imgs, H, W, 2], F32, kind="ExternalInput")
    NCC = 9 * n_imgs + n_rb + 10
    CC = nc.dram_tensor("CC", [128, NCC], F32, kind="ExternalInput")
    GX = nc.dram_tensor("GX", [128, 1024], F32, kind="ExternalInput")
    SM = nc.dram_tensor("SM", [128, 384], BF16, kind="ExternalInput")
    OUT = nc.dram_tensor("OUT", [n_imgs, H, W, 2], F32, kind="ExternalOutput")

    NBP = max(1, NBR * n_imgs)  # band partitions

    with TileContext(nc) as tc:
        with (
            tc.tile_pool(name="stat", bufs=1) as pstat,
            tc.tile_pool(name="pin", bufs=3) as pin,
            tc.tile_pool(name="ptmp", bufs=2) as ptmp,
            tc.tile_pool(name="pwarp", bufs=2) as pwarp,
            tc.tile_pool(name="pband", bufs=1) as pband,
            tc.tile_pool(name="pps", bufs=2, space="PSUM") as pps,
        ):
            gx2 = pstat.tile([128, 1024], F32)
            nc.sync.dma_start(gx2[:], GX[:])
            cc = pstat.tile([128, NCC], F32)
            nc.sync.dma_start(cc[:], CC[:])
            sm = pstat.tile([128, 384], BF16)
            nc.sync.dma_start(sm[:], SM[:])
            cmth = pstat.tile([128, 1], F32)
            nc.gpsimd.memset(cmth[:], -1022.0)

            def cC(j):  # [128,1] column of cc
                return cc[:, j : j + 1]

            # ------------ warp chain: 4 EX/EY-free terms (+EX if asked) -----
            def warp_chain(pool, tag, P, fdims, t2x, t2y, bimg, with_ex):
                """wm = -0.1*warped*[t2x>=C1][t2y>=C1] into a fresh tile.
                with_ex=False drops the EX and EY basis terms (caller must
                patch the strips where they are nonzero)."""
                cof = 9 * n_imgs + n_rb + 1

                def col(k):
                    c = cC(cof + k) if bimg is None else cC(9 * bimg + k)
                    return c[:P]

                shp = [P] + list(fdims)

                def T(nm, bufs=1):
                    return pool.tile(shp, F32, tag=f"{tag}{nm}",
                                     name=f"{tag}{nm}", bufs=bufs)

                if with_ex:
                    ex = T("ex")
                    nc.scalar.activation(ex[:], t2x, ACTF.Relu,
                                         bias=cmth[:P], scale=1.0)
                pt = T("pt")
                nc.scalar.activation(pt[:], t2x, ACTF.Identity,
                                     bias=col(0), scale=col(1))
                qt = T("qt")
                nc.scalar.activation(qt[:], t2x, ACTF.Identity,
                                     bias=col(3), scale=col(4))
                if with_ex:
                    eg = T("eg", bufs=2)
                    nc.scalar.activation(eg[:], ex[:], ACTF.Identity,
                                         bias=0.0, scale=col(2))
                    nc.vector.tensor_tensor(pt[:], pt[:], eg[:], ALU.add)
                    eg2 = T("eg", bufs=2)
                    nc.scalar.activation(eg2[:], ex[:], ACTF.Identity,
                                         bias=0.0, scale=col(5))
                    nc.vector.tensor_tensor(qt[:], qt[:], eg2[:], ALU.add)
                nc.vector.tensor_tensor(qt[:], t2y, qt[:], ALU.mult)
                nc.vector.tensor_tensor(pt[:], pt[:], qt[:], ALU.add)
                return pt

            def apply_masks(pt, t2x, t2y):
                nc.vector.scalar_tensor_tensor(pt, t2x, C1, pt,
                                               ALU.is_ge, ALU.mult)
                nc.vector.scalar_tensor_tensor(pt, t2y, C1, pt,
                                               ALU.is_ge, ALU.mult)

            # ---------------- band strip (rows hz..255, all imgs) -----------
            if NBC > 0:
                bi1 = pband.tile([NBP, 512], F32)
                bi1r = pband.tile([NBP, 512], F32)
                bi2 = pband.tile([NBP, 512], F32)
                bfl = pband.tile([NBP, 512, 2], F32)
                for b in range(n_imgs):
                    bsl = slice(NBR * b, NBR * (b + 1))
                    nc.sync.dma_start(bi1[bsl, :], I1[b, hz : hz + NBR, :])
                    nc.sync.dma_start(
                        bi1r[bsl, :], I1[b, hz + 1 : hz + 1 + NBR, :]
                    )
                    nc.sync.dma_start(bi2[bsl, :], I2[b, hz : hz + NBR, :])
                    nc.sync.dma_start(
                        bfl[bsl, :, :], FL[b, hz : hz + NBR, :, :]
                    )
                bu = bfl[:, :, 0]
                bv = bfl[:, :, 1]
                bt2x = pband.tile([NBP, 512], F32)
                nc.vector.tensor_tensor(bt2x[:], bu, gx2[:NBP, 0:512],
                                        ALU.add)
                bt2y = pband.tile([NBP, 512], F32)
                nc.scalar.activation(
                    bt2y[:], bv, ACTF.Identity,
                    bias=cC(9 * n_imgs + n_rb)[:NBP], scale=1.0,
                )
                wmB = warp_chain(pband, "bnd", NBP, [512], bt2x[:], bt2y[:],
                                 None, with_ex=True)
                apply_masks(wmB[:], bt2x[:], bt2y[:])
                bdt = pband.tile([NBP, 512], F32)
                nc.vector.scalar_tensor_tensor(bdt[:], bi2[:], 0.1, wmB[:],
                                               ALU.mult, ALU.add)
                bg1 = pband.tile([NBP, 512], F32)
                nc.vector.tensor_tensor(bg1[:], bi1r[:], bi1[:], ALU.subtract)
                bg2 = pband.tile([NBP, 512], F32)
                nc.vector.tensor_tensor(
                    bg2[:, 0:511], bi1[:, 1:512], bi1[:, 0:511], ALU.subtract
                )
                nc.gpsimd.memset(bg2[:, 511:512], 0.0)
                nc.gpsimd.tensor_tensor(bg1[:], bdt[:], bg1[:], ALU.mult)
                nc.vector.tensor_tensor(bu, bu, bg1[:], ALU.add)
                nc.gpsimd.tensor_tensor(bg2[:], bdt[:], bg2[:], ALU.mult)
                nc.vector.tensor_tensor(bv, bv, bg2[:], ALU.add)

            # ---------------- per image ------------------------------------
            NW = n_rb * 512
            for b in range(n_imgs):
                i1 = pin.tile([128, NW], F32, tag="i1")
                nc.sync.dma_start(
                    i1[:].rearrange("p (rb w) -> p rb w", rb=n_rb),
                    I1[b].rearrange("(rb p) w -> p rb w", p=128),
                )
                i2 = pin.tile([128, NW], F32, tag="i2")
                nc.sync.dma_start(
                    i2[:].rearrange("p (rb w) -> p rb w", rb=n_rb),
                    I2[b].rearrange("(rb p) w -> p rb w", p=128),
                )
                fl = pin.tile([128, NW, 2], F32, tag="fl")
                nc.sync.dma_start(
                    fl[:].rearrange("p (rb w) c -> p rb w c", rb=n_rb),
                    FL[b].rearrange("(rb p) w c -> p rb w c", p=128),
                )

                # bf16 copy of I1 (feeds PE and the column gradient)
                i1b = ptmp.tile([128, NW], BF16, tag="i1b")
                nc.scalar.activation(i1b[:], i1[:], ACTF.Identity, bias=0.0,
                                     scale=1.0)

                # PE: ps = I1[r+1]-I1[r] into PSUM (row 511 -> 0)
                ps = pps.tile([128, NW], F32, tag="ps")
                for rb in range(n_rb):
                    dst = ps[:, rb * 512 : (rb + 1) * 512]
                    rhs = i1b[:, rb * 512 : (rb + 1) * 512]
                    if rb < n_rb - 1:
                        nc.tensor.matmul(dst, sm[:, 0:128], rhs,
                                         start=True, stop=False)
                        rhs2 = i1b[:, (rb + 1) * 512 : (rb + 2) * 512]
                        nc.tensor.matmul(dst, sm[:, 128:256], rhs2,
                                         start=False, stop=True)
                    else:
                        nc.tensor.matmul(dst, sm[:, 256:384], rhs,
                                         start=True, stop=True)

                # dt0 = 0.1*I2 (bf16), g2 = col-gradient of I1 (bf16, 2x DVE)
                dt0 = ptmp.tile([128, NW], BF16, tag="dt0")
                nc.scalar.activation(dt0[:], i2[:], ACTF.Identity, bias=0.0,
                                     scale=0.1)
                g2 = ptmp.tile([128, NW], BF16, tag="g2")
                nc.vector.tensor_tensor(g2[:, 0 : NW - 1], i1b[:, 1:NW],
                                        i1b[:, 0 : NW - 1], ALU.subtract)
                g2r = g2[:].rearrange("p (r w) -> p r w", r=n_rb)
                nc.gpsimd.memset(g2r[:, :, 511:512], 0.0)

                # ---- warp quadrant: rb 2,3  cols [wz,512) ----
                flv = fl[:].rearrange("p (r w) c -> p r w c", r=n_rb)
                ur = flv[:, 2:4, wz:, 0]
                vr = flv[:, 2:4, wz:, 1]
                dt0v = dt0[:].rearrange("p (r w) -> p r w", r=n_rb)[
                    :, 2:4, wz:
                ]
                gxf = gx2[:].rearrange("p (r w) -> p r w", r=2)[:, :, wz:]

                t2x = pwarp.tile([128, 2, WF], F32, tag="t2x")
                nc.gpsimd.tensor_tensor(t2x[:], ur, gxf, ALU.add)
                t2y = pwarp.tile([128, 2, WF], F32, tag="t2y")
                for rbl in range(2):
                    nc.scalar.activation(
                        t2y[:, rbl, :], vr[:, rbl, :], ACTF.Identity,
                        bias=cC(9 * n_imgs + 2 + rbl), scale=1.0,
                    )
                wm = warp_chain(pwarp, "w", 128, [2, WF], t2x[:], t2y[:], b,
                                with_ex=False)

                def fcol(k):
                    return cC(9 * b + k)

                # EX strip: += EX*(F02 + F12*t2y) on cols [wze,512)
                if WE > 0:
                    exs = pwarp.tile([128, 2, WE], F32, tag="exs")
                    nc.scalar.activation(exs[:], t2x[:, :, XS:], ACTF.Relu,
                                         bias=cmth[:], scale=1.0)
                    e1 = pwarp.tile([128, 2, WE], F32, tag="e1")
                    nc.scalar.activation(e1[:], t2y[:, :, XS:],
                                         ACTF.Identity, bias=fcol(2),
                                         scale=fcol(5))
                    nc.vector.tensor_tensor(e1[:], e1[:], exs[:], ALU.mult)
                    nc.vector.tensor_tensor(wm[:, :, XS:], wm[:, :, XS:],
                                            e1[:], ALU.add)
                # EY strip: += EY*(F20 + F21*t2x + F22*EX) on rows
                # [hze,512) = inside partitions [96,128) of rb 3 (rows below
                # hze contribute EY == 0 exactly, so the wide slice is safe)
                if hze < 512:
                    eys = pwarp.tile([128, WF], F32, tag="eys")
                    nc.scalar.activation(eys[96:128, :], t2y[96:128, 1, :],
                                         ACTF.Relu, bias=cmth[96:128],
                                         scale=1.0)
                    e2 = pwarp.tile([128, WF], F32, tag="e2")
                    nc.scalar.activation(e2[96:128, :], t2x[96:128, 1, :],
                                         ACTF.Identity,
                                         bias=fcol(6)[96:128],
                                         scale=fcol(7)[96:128])
                    if WE > 0:
                        egc = pwarp.tile([128, WE], F32, tag="egc")
                        nc.scalar.activation(egc[96:128, :],
                                             exs[96:128, 1, :],
                                             ACTF.Identity, bias=0.0,
                                             scale=fcol(8)[96:128])
                        nc.vector.tensor_tensor(e2[96:128, XS:],
                                                e2[96:128, XS:],
                                                egc[96:128, :], ALU.add)
                    nc.vector.tensor_tensor(e2[96:128, :], e2[96:128, :],
                                            eys[96:128, :], ALU.mult)
                    nc.vector.tensor_tensor(wm[96:128, 1, :],
                                            wm[96:128, 1, :],
                                            e2[96:128, :], ALU.add)

                apply_masks(wm[:], t2x[:], t2y[:])
                # dt (both paths): dt0 += wm on the quadrant
                nc.vector.tensor_tensor(dt0v, dt0v, wm[:], ALU.add)

                # ---- full-width updates ----
                # m1 = dt * (I1[r+1]-I1[r])   (PSUM src => DVE; onto i2)
                nc.vector.tensor_tensor(i2[:], dt0[:], ps[:], ALU.mult)
                flu = fl[:, :, 0]
                nc.gpsimd.tensor_tensor(flu, flu, i2[:], ALU.add)
                # m2 = dt * g2 (bf16 2x, onto g2)
                nc.vector.tensor_tensor(g2[:], dt0[:], g2[:], ALU.mult)
                flv2 = fl[:, :, 1]
                nc.gpsimd.tensor_tensor(flv2, flv2, g2[:], ALU.add)

                # patch band rows (overwrites the zero-branch values there)
                if NBC > 0:
                    nc.scalar.dma_start(
                        flv[hz - 128 : hz - 128 + NBC, 1, :, :],
                        bfl[NBR * b : NBR * b + NBC, :, :],
                    )

                nc.scalar.dma_start(
                    OUT[b].rearrange("(rb p) w c -> p rb w c", p=128),
                    fl[:].rearrange("p (rb w) c -> p rb w c", rb=n_rb),
                )
    if legalize:
        legalize_single_wait(nc)
    return nc


# ---------------------------------------------------------------------------
# Post-pass: this walrus build encodes a single sync-wait slot per TPB
# instruction. Tile's sem assignment can emit 2+ waits on one instruction;
# hoist all but the last wait onto same-engine EventSemaphore carriers placed
# immediately before it (the sequencer then waits sequentially, which is
# semantically identical).
def legalize_single_wait(nc):
    import bass_rust

    capped = {
        mybir.EngineType.Activation,
        mybir.EngineType.DVE,
        mybir.EngineType.Pool,
        mybir.EngineType.PE,
        mybir.EngineType.SP,
    }
    exempt = {"EventSemaphore", "NoOp", "TriggerDma"}
    n = 0
    for fn in nc.m.functions:
        for blk in fn.blocks:
            insts = blk.instructions  # live list
            rebuilt = []
            changed = False
            for inst in list(insts):
                si = inst.sync_info
                waits = list(si.on_wait) if si is not None else []
                if (
                    len(waits) > 1
                    and inst.engine in capped
                    and str(inst.opcode) not in exempt
                ):
                    for w in waits[:-1]:
                        ev = mybir.InstEventSemaphore(
                            name=f"waitcarrier_{inst.name}_{n}", ins=[], outs=[]
                        )
                        ev.engine = inst.engine
                        ev.sync_info = bass_rust.SyncInfo(
                            on_wait=[w], on_update=[]
                        )
                        rebuilt.append(ev)
                        n += 1
                    inst.sync_info = bass_rust.SyncInfo(
                        on_wait=[waits[-1]], on_update=list(si.on_update)
                    )
                    changed = True
                rebuilt.append(inst)
            if changed:
                insts[:] = rebuilt
    return n


def _img_consts(P3: np.ndarray) -> np.ndarray:
    """9 warp consts F[i,j] (row-major) for one image's 3x3 corner P3[y,x].

    warped = sum_ij F'[i,j]*ay_i*ax_j, ax=(1,t2x,relu(t2x-1022)),
    ay=(1,t2y,relu(t2y-1022));  F = -0.1*F'.
    """
    P = P3.astype(np.float64)
    E = np.stack([P[:, 0], P[:, 1] - P[:, 0], P[:, 2] - P[:, 1]], axis=1)
    D = np.stack([E[0], E[1] - E[0], E[2] - E[1]], axis=0)
    r = 1.0 / 511.0
    Mx = np.array([[1.0, 0.0, 0.0], [-1.0, r, -r], [0.0, 0.0, r]])
    F = -0.1 * (Mx.T @ D @ Mx)
    return F.reshape(-1).astype(np.float32)


def host_consts(I1c: np.ndarray, n_rb: int = 4, hz: int = 253) -> np.ndarray:
    """Per-image folded warp coefficients + per-partition 2*h columns.

    I1c: [n_imgs, H, W] float32.  Returns [128, 9*n_imgs + n_rb + 10] f32.
    Per image b, cols 9*b+3*i+j hold F[i,j].  Col 9n+rb: 2*(128*rb+p).
    Col 9n+n_rb: band 2*h.  Cols 9n+n_rb+1..+9: band-partition-layout
    consts (partition NBR*b+r holds image b's values).
    """
    f = np.float32
    n_imgs = I1c.shape[0]
    cc = np.zeros((128, 9 * n_imgs + n_rb + 10), dtype=np.float32)
    allc = np.zeros((n_imgs, 9), dtype=np.float32)
    for b in range(n_imgs):
        allc[b] = _img_consts(I1c[b, 0:3, 0:3])
        cc[:, 9 * b : 9 * b + 9] = allc[b][None, :]
    p = np.arange(128, dtype=np.float32)
    for rb in range(n_rb):
        cc[:, 9 * n_imgs + rb] = f(2.0) * (f(128.0 * rb) + p)
    # band columns (NBR = 257-hz rows per image)
    base = 9 * n_imgs + n_rb
    nbr = 257 - hz
    for b in range(n_imgs):
        for r in range(nbr):
            pp = nbr * b + r
            if pp < 128:
                cc[pp, base] = f(2.0) * f(hz + r)
                cc[pp, base + 1 : base + 10] = allc[b]
    return cc


def host_gx() -> np.ndarray:
    w2 = (np.float32(2.0) * np.arange(512, dtype=np.float32)).astype(np.float32)
    return np.tile(w2, (128, 2)).astype(np.float32)


def host_sm() -> np.ndarray:
    """[128, 384] bf16: cols 0:128 = shift lhsT S (S[k,m]: +1 at k=m+1,
    -1 at k=m), cols 128:256 = patch lhsT (+1 at k=0, m=127), cols
    256:384 = S with column 127 zeroed (dy row 511 must be exactly 0)."""
    sm = np.zeros((128, 384), dtype=np.float32)
    for m in range(128):
        sm[m, m] = -1.0
        if m + 1 < 128:
            sm[m + 1, m] = 1.0
    sm[0, 128 + 127] = 1.0
    sm[:, 256:384] = sm[:, 0:128]
    sm[127, 256 + 127] = 0.0
    return sm.astype(ml_dtypes.bfloat16)


_NC = None
_NC_KEY = None


def _get_nc(wz, hz, wze, hze):
    global _NC, _NC_KEY
    if _NC is None or _NC_KEY != (wz, hz, wze, hze):
        _NC = build_nc(4, 4, wz=wz, hz=hz, wze=wze, hze=hze)
        _NC_KEY = (wz, hz, wze, hze)
    return _NC


def _splits(flow):
    umax = float(max(flow[..., 0].max(), 0.0))
    vmax = float(max(flow[..., 1].max(), 0.0))
    # first col/row where 2*x + d can reach 511.0 (f32-exact threshold)
    wz = int(min(256, max(1, (511.0 - umax) // 2 + 1)))
    hz = int(min(256, max(225, (511.0 - vmax) // 2 + 1)))
    assert np.float32(2.0 * (wz - 1)) + np.float32(umax) < np.float32(511.0)
    assert np.float32(2.0 * (hz - 1)) + np.float32(vmax) < np.float32(511.0)
    # first col/row where 2*x + d can reach 1022.0 (EX/EY strips)
    wze = int(min(512, max(wz + 1, (1022.0 - umax) // 2 + 1)))
    hze = int(min(512, max(481, (1022.0 - vmax) // 2 + 1)))
    assert wze == 512 or (
        np.float32(2.0 * (wze - 1)) + np.float32(umax) < np.float32(1022.0)
    )
    assert hze == 512 or (
        np.float32(2.0 * (hze - 1)) + np.float32(vmax) < np.float32(1022.0)
    )
    return wz, hz, wze, hze


def _make_in_maps(I1, I2, flow, wz, hz, n_cores=8):
    per = I1.shape[0] // n_cores
    gx = host_gx()
    sm = host_sm()
    in_maps = []
    for c in range(n_cores):
        sl = slice(c * per, (c + 1) * per)
        i1c = np.ascontiguousarray(I1[sl, :, :, 0], dtype=np.float32)
        in_maps.append(
            {
                "I1": i1c,
                "I2": np.ascontiguousarray(I2[sl, :, :, 0], dtype=np.float32),
                "FL": np.ascontiguousarray(flow[sl], dtype=np.float32),
                "CC": host_consts(i1c, 4, hz),
                "GX": gx,
                "SM": sm,
            }
        )
    return in_maps


def run(I1, I2, flow, trace=False, **kw):
    wz, hz, wze, hze = _splits(np.asarray(flow))
    nc = _get_nc(wz, hz, wze, hze)
    in_maps = _make_in_maps(I1, I2, flow, wz, hz)
    res = run_bass_kernel_spmd(nc, in_maps, list(range(8)), trace=trace, **kw)
    out = np.concatenate([r["OUT"] for r in res.results], axis=0)
    return out, res


def kernel(I1, I2, flow):
    out, _ = run(I1, I2, flow)
    return out.astype(np.float32)


# revision 10
# speedup vs baseline: 1.3195x; 1.0435x over previous
"""Trainium2 Bass kernel for nn_DataTermLayer (data-term update of optical-flow).

Key observation: the reference's bilinear warp feeds *normalized* coords in
[-1, 1] straight into a pixel-space sampler, so after clipping the gather
only ever touches I1[b, 0:3, 0:3]. The whole layer reduces to elementwise
math plus 9 per-image scalars:

  t2x = u + 2*w ; t2y = v + 2*h          (pre-division coords, f32-exact)
  x   = t2x/511 - 1 ; y = t2y/511 - 1
  warped = [x>=0][y>=0] * bilinear3x3(P, x, y)
  dt    = 0.1*(I2 - warped)
  out_u = u + dt*(I1[h+1,w]-I1[h,w]) ; out_v = v + dt*(I1[h,w+1]-I1[h,w])

Structure (2e-2 rel tolerance; measured ~1e-4):
  * I1 is cast once to bf16; the row gradient comes from the idle PE as a
    +-1 bidiagonal bf16 shift-matmul into PSUM (kills the baseline's
    duplicate shifted-I1 HBM load and the DVE subtract), and the column
    gradient is a 2x-rate bf16 DVE subtract.
  * dt0 = 0.1*I2 (bf16, ACT engine).  warped is expanded EXACTLY in the
    basis (1,t2x,EX)x(1,t2y,EY), EX=relu(t2x-1022): on the bottom-right
    warp quadrant only the 4 EX/EY-free terms run full-size; the EX terms
    live only in the last ~3 columns and EY in the last ~3 rows, patched
    by tiny strip ops (the Y strip runs on partitions 96:128 where
    EY==0 rows self-cancel).  Masks are f32-exact compares vs 511 in
    pre-division space (warped == 0 wherever 2w+u < 511 or 2h+v < 511).
  * A 3-row "band" strip (rows hz..255 of all images in one tile) redoes
    the rows adjacent to the half boundary with the full chain, as in
    the baseline.
  * The flow updates run on the GpSimd engine, everything PSUM-touching
    on DVE, single-source ops on ACT; output DMAs trigger from the ACT
    queue so they never block the SP input-DMA stream.

Sharding: pure data-parallel, 4 images per core across 8 cores.
"""
import sys

sys.path.insert(0, "/opt/trn_rl_repo")

import numpy as np
import ml_dtypes

import concourse.bass as bass
import concourse.mybir as mybir
from concourse.bass_utils import run_bass_kernel_spmd
from concourse.tile import TileContext

F32 = mybir.dt.float32
BF16 = mybir.dt.bfloat16
ALU = mybir.AluOpType
ACTF = mybir.ActivationFunctionType

C1 = 511.0  # min f32 t with fl(t/511) >= 1  (verified exhaustively)


def build_nc(n_imgs: int = 4, n_rb: int = 4, wz: int = 253, hz: int = 253,
             wze: int = 509, hze: int = 509, legalize: bool = True):
    """One NeuronCore program: n_imgs images of [512, 512].

    wz/hz: first col/row where the warp can be nonzero (t2 >= 511
    reachable).  wze/hze: first col/row where EX/EY (t2 >= 1022) can be
    nonzero.
    """
    assert n_rb == 4 and 225 <= hz <= 256 and 0 < wz <= 256
    assert wz < wze <= 512 and max(hz, 480) < hze <= 512
    W = 512
    H = n_rb * 128
    NBC = 256 - hz  # band compute rows per image (rows hz..255)
    NBR = NBC + 1   # band rows loaded per image (+1 for the row-shift grad)
    WF = W - wz     # warp-math columns
    WE = W - wze    # EX strip columns
    XS = wze - wz   # EX strip offset inside the warp quadrant
    nc = bass.Bass()

    I1 = nc.dram_tensor("I1", [n_imgs, H, W], F32, kind="ExternalInput")
    I2 = nc.dram_tensor("I2", [n_imgs, H, W], F32, kind="ExternalInput")
    FL = nc.dram_tensor("FL", [n_imgs, H, W, 2], F32, kind="ExternalInput")
    NCC = 9 * n_imgs + n_rb + 10
    CC = nc.dram_tensor("CC", [128, NCC], F32, kind="ExternalInput")
    GX = nc.dram_tensor("GX", [128, 1024], F32, kind="ExternalInput")
    SM = nc.dram_tensor("SM", [128, 384], BF16, kind="ExternalInput")
    OUT = nc.dram_tensor("OUT", [n_imgs, H, W, 2], F32, kind="ExternalOutput")

    NBP = max(1, NBR * n_imgs)  # band partitions

    with TileContext(nc) as tc:
        with (
            tc.tile_pool(name="stat", bufs=1) as pstat,
            tc.tile_pool(name="pin", bufs=3) as pin,
            tc.tile_pool(name="ptmp", bufs=2) as ptmp,
            tc.tile_pool(name="pwarp", bufs=2) as pwarp,
            tc.tile_pool(name="pband", bufs=1) as pband,
            tc.tile_pool(name="pps", bufs=2, space="PSUM") as pps,
        ):
            gx2 = pstat.tile([128, 1024], F32)
            nc.sync.dma_start(gx2[:], GX[:])
            cc = pstat.tile([128, NCC], F32)
            nc.sync.dma_start(cc[:], CC[:])
            sm = pstat.tile([128, 384], BF16)
            nc.sync.dma_start(sm[:], SM[:])
            cmth = pstat.tile([128, 1], F32)
            nc.gpsimd.memset(cmth[:], -1022.0)

            def cC(j):  # [128,1] column of cc
                return cc[:, j : j + 1]

            # ------------ warp chain: 4 EX/EY-free terms (+EX if asked) -----
            def warp_chain(pool, tag, P, fdims, t2x, t2y, bimg, with_ex):
                """wm = -0.1*warped*[t2x>=C1][t2y>=C1] into a fresh tile.
                with_ex=False drops the EX and EY basis terms (caller must
                patch the strips where they are nonzero)."""
                cof = 9 * n_imgs + n_rb + 1

                def col(k):
                    c = cC(cof + k) if bimg is None else cC(9 * bimg + k)
                    return c[:P]

                shp = [P] + list(fdims)

                def T(nm, bufs=1):
                    return pool.tile(shp, F32, tag=f"{tag}{nm}",
                                     name=f"{tag}{nm}", bufs=bufs)

                if with_ex:
                    ex = T("ex")
                    nc.scalar.activation(ex[:], t2x, ACTF.Relu,
                                         bias=cmth[:P], scale=1.0)
                pt = T("pt")
                nc.scalar.activation(pt[:], t2x, ACTF.Identity,
                                     bias=col(0), scale=col(1))
                qt = T("qt")
                nc.scalar.activation(qt[:], t2x, ACTF.Identity,
                                     bias=col(3), scale=col(4))
                if with_ex:
                    eg = T("eg", bufs=2)
                    nc.scalar.activation(eg[:], ex[:], ACTF.Identity,
                                         bias=0.0, scale=col(2))
                    nc.vector.tensor_tensor(pt[:], pt[:], eg[:], ALU.add)
                    eg2 = T("eg", bufs=2)
                    nc.scalar.activation(eg2[:], ex[:], ACTF.Identity,
                                         bias=0.0, scale=col(5))
                    nc.vector.tensor_tensor(qt[:], qt[:], eg2[:], ALU.add)
                nc.vector.tensor_tensor(qt[:], t2y, qt[:], ALU.mult)
                nc.vector.tensor_tensor(pt[:], pt[:], qt[:], ALU.add)
                return pt

            def apply_masks(pt, t2x, t2y):
                nc.vector.scalar_tensor_tensor(pt, t2x, C1, pt,
                                               ALU.is_ge, ALU.mult)
                nc.vector.scalar_tensor_tensor(pt, t2y, C1, pt,
                                               ALU.is_ge, ALU.mult)

            # ---------------- band strip (rows hz..255, all imgs) -----------
            if NBC > 0:
                bi1 = pband.tile([NBP, 512], F32)
                bi1r = pband.tile([NBP, 512], F32)
                bi2 = pband.tile([NBP, 512], F32)
                bfl = pband.tile([NBP, 512, 2], F32)
                for b in range(n_imgs):
                    bsl = slice(NBR * b, NBR * (b + 1))
                    nc.sync.dma_start(bi1[bsl, :], I1[b, hz : hz + NBR, :])
                    nc.sync.dma_start(
                        bi1r[bsl, :], I1[b, hz + 1 : hz + 1 + NBR, :]
                    )
                    nc.sync.dma_start(bi2[bsl, :], I2[b, hz : hz + NBR, :])
                    nc.sync.dma_start(
                        bfl[bsl, :, :], FL[b, hz : hz + NBR, :, :]
                    )
                bu = bfl[:, :, 0]
                bv = bfl[:, :, 1]
                bt2x = pband.tile([NBP, 512], F32)
                nc.vector.tensor_tensor(bt2x[:], bu, gx2[:NBP, 0:512],
                                        ALU.add)
                bt2y = pband.tile([NBP, 512], F32)
                nc.scalar.activation(
                    bt2y[:], bv, ACTF.Identity,
                    bias=cC(9 * n_imgs + n_rb)[:NBP], scale=1.0,
                )
                wmB = warp_chain(pband, "bnd", NBP, [512], bt2x[:], bt2y[:],
                                 None, with_ex=True)
                apply_masks(wmB[:], bt2x[:], bt2y[:])
                bdt = pband.tile([NBP, 512], F32)
                nc.vector.scalar_tensor_tensor(bdt[:], bi2[:], 0.1, wmB[:],
                                               ALU.mult, ALU.add)
                bg1 = pband.tile([NBP, 512], F32)
                nc.vector.tensor_tensor(bg1[:], bi1r[:], bi1[:], ALU.subtract)
                bg2 = pband.tile([NBP, 512], F32)
                nc.vector.tensor_tensor(
                    bg2[:, 0:511], bi1[:, 1:512], bi1[:, 0:511], ALU.subtract
                )
                nc.gpsimd.memset(bg2[:, 511:512], 0.0)
                nc.gpsimd.tensor_tensor(bg1[:], bdt[:], bg1[:], ALU.mult)
                nc.vector.tensor_tensor(bu, bu, bg1[:], ALU.add)
                nc.gpsimd.tensor_tensor(bg2[:], bdt[:], bg2[:], ALU.mult)
                nc.vector.tensor_tensor(bv, bv, bg2[:], ALU.add)

            # ---------------- per image ------------------------------------
            # All input-DMA triggers issue up front (SP: i1/i2, PE: fl) so
            # descriptor generation never blocks behind compute waits; the
            # output triggers share the otherwise-idle SP queue.
            NW = n_rb * 512
            HWD = NW // 2  # free elems of one half (rb pair)
            i1s, i2s, fls = [], [], []
            for b in range(n_imgs):
                i1 = pin.tile([128, NW], F32, tag="i1", bufs=2)
                nc.sync.dma_start(
                    i1[:].rearrange("p (rb w) -> p rb w", rb=n_rb),
                    I1[b].rearrange("(rb p) w -> p rb w", p=128),
                )
                i2 = pin.tile([128, NW], F32, tag="i2", bufs=4)
                nc.sync.dma_start(
                    i2[:].rearrange("p (rb w) -> p rb w", rb=n_rb),
                    I2[b].rearrange("(rb p) w -> p rb w", p=128),
                )
                fl = pin.tile([128, NW, 2], F32, tag="fl", bufs=4)
                nc.sync.dma_start(
                    fl[:].rearrange("p (rb w) c -> p rb w c", rb=n_rb),
                    FL[b].rearrange("(rb p) w c -> p rb w c", p=128),
                )
                i1s.append(i1)
                i2s.append(i2)
                fls.append(fl)

            for b in range(n_imgs):
                i1, i2, fl = i1s[b], i2s[b], fls[b]

                # bf16 copy of I1 (feeds PE and the column gradient)
                i1b = ptmp.tile([128, NW], BF16, tag="i1b")
                nc.scalar.activation(i1b[:], i1[:], ACTF.Identity, bias=0.0,
                                     scale=1.0)

                # PE: ps = I1[r+1]-I1[r] into PSUM (row 511 -> 0)
                ps = pps.tile([128, NW], F32, tag="ps")
                for rb in range(n_rb):
                    dst = ps[:, rb * 512 : (rb + 1) * 512]
                    rhs = i1b[:, rb * 512 : (rb + 1) * 512]
                    if rb < n_rb - 1:
                        nc.tensor.matmul(dst, sm[:, 0:128], rhs,
                                         start=True, stop=False)
                        rhs2 = i1b[:, (rb + 1) * 512 : (rb + 2) * 512]
                        nc.tensor.matmul(dst, sm[:, 128:256], rhs2,
                                         start=False, stop=True)
                    else:
                        nc.tensor.matmul(dst, sm[:, 256:384], rhs,
                                         start=True, stop=True)

                # dt0 = 0.1*I2 (bf16), g2 = col-gradient of I1 (bf16, 2x DVE)
                dt0 = ptmp.tile([128, NW], BF16, tag="dt0")
                nc.scalar.activation(dt0[:], i2[:], ACTF.Identity, bias=0.0,
                                     scale=0.1)
                g2 = ptmp.tile([128, NW], BF16, tag="g2")
                nc.vector.tensor_tensor(g2[:, 0 : NW - 1], i1b[:, 1:NW],
                                        i1b[:, 0 : NW - 1], ALU.subtract)
                g2r = g2[:].rearrange("p (r w) -> p r w", r=n_rb)
                nc.gpsimd.memset(g2r[:, :, 511:512], 0.0)

                flu = fl[:, :, 0]
                flv2 = fl[:, :, 1]
                flv = fl[:].rearrange("p (r w) c -> p r w c", r=n_rb)

                # ---- top half (rb 0,1): warp-free, store early ----
                tp = slice(0, HWD)
                nc.vector.tensor_tensor(i2[:, tp], dt0[:, tp], ps[:, tp],
                                        ALU.mult)
                nc.gpsimd.tensor_tensor(flu[:, tp], flu[:, tp], i2[:, tp],
                                        ALU.add)
                nc.vector.tensor_tensor(g2[:, tp], dt0[:, tp], g2[:, tp],
                                        ALU.mult)
                nc.gpsimd.tensor_tensor(flv2[:, tp], flv2[:, tp], g2[:, tp],
                                        ALU.add)
                if NBC > 0:
                    nc.sync.dma_start(
                        flv[hz - 128 : hz - 128 + NBC, 1, :, :],
                        bfl[NBR * b : NBR * b + NBC, :, :],
                    )
                nc.sync.dma_start(
                    OUT[b, 0:256].rearrange("(rb p) w c -> p rb w c", p=128),
                    fl[:, tp, :].rearrange("p (rb w) c -> p rb w c", rb=2),
                )

                # ---- bottom half: warp quadrant rb 2,3 cols [wz,512) ----
                ur = flv[:, 2:4, wz:, 0]
                vr = flv[:, 2:4, wz:, 1]
                dt0v = dt0[:].rearrange("p (r w) -> p r w", r=n_rb)[
                    :, 2:4, wz:
                ]
                gxf = gx2[:].rearrange("p (r w) -> p r w", r=2)[:, :, wz:]

                t2x = pwarp.tile([128, 2, WF], F32, tag="t2x")
                nc.gpsimd.tensor_tensor(t2x[:], ur, gxf, ALU.add)
                t2y = pwarp.tile([128, 2, WF], F32, tag="t2y")
                for rbl in range(2):
                    nc.scalar.activation(
                        t2y[:, rbl, :], vr[:, rbl, :], ACTF.Identity,
                        bias=cC(9 * n_imgs + 2 + rbl), scale=1.0,
                    )
                wm = warp_chain(pwarp, "w", 128, [2, WF], t2x[:], t2y[:], b,
                                with_ex=False)

                def fcol(k):
                    return cC(9 * b + k)

                # EX strip: += EX*(F02 + F12*t2y) on cols [wze,512)
                if WE > 0:
                    exs = pwarp.tile([128, 2, WE], F32, tag="exs")
                    nc.scalar.activation(exs[:], t2x[:, :, XS:], ACTF.Relu,
                                         bias=cmth[:], scale=1.0)
                    e1 = pwarp.tile([128, 2, WE], F32, tag="e1")
                    nc.scalar.activation(e1[:], t2y[:, :, XS:],
                                         ACTF.Identity, bias=fcol(2),
                                         scale=fcol(5))
                    nc.vector.tensor_tensor(e1[:], e1[:], exs[:], ALU.mult)
                    nc.vector.tensor_tensor(wm[:, :, XS:], wm[:, :, XS:],
                                            e1[:], ALU.add)
                # EY strip: += EY*(F20 + F21*t2x + F22*EX) on rows
                # [hze,512) = inside partitions [96,128) of rb 3 (rows below
                # hze contribute EY == 0 exactly, so the wide slice is safe)
                if hze < 512:
                    eys = pwarp.tile([128, WF], F32, tag="eys")
                    nc.scalar.activation(eys[96:128, :], t2y[96:128, 1, :],
                                         ACTF.Relu, bias=cmth[96:128],
                                         scale=1.0)
                    e2 = pwarp.tile([128, WF], F32, tag="e2")
                    nc.scalar.activation(e2[96:128, :], t2x[96:128, 1, :],
                                         ACTF.Identity,
                                         bias=fcol(6)[96:128],
                                         scale=fcol(7)[96:128])
                    if WE > 0:
                        egc = pwarp.tile([128, WE], F32, tag="egc")
                        nc.scalar.activation(egc[96:128, :],
                                             exs[96:128, 1, :],
                                             ACTF.Identity, bias=0.0,
                                             scale=fcol(8)[96:128])
                        nc.vector.tensor_tensor(e2[96:128, XS:],
                                                e2[96:128, XS:],
                                                egc[96:128, :], ALU.add)
                    nc.vector.tensor_tensor(e2[96:128, :], e2[96:128, :],
                                            eys[96:128, :], ALU.mult)
                    nc.vector.tensor_tensor(wm[96:128, 1, :],
                                            wm[96:128, 1, :],
                                            e2[96:128, :], ALU.add)

                apply_masks(wm[:], t2x[:], t2y[:])
                # dt (both paths): dt0 += wm on the quadrant
                nc.gpsimd.tensor_tensor(dt0v, dt0v, wm[:], ALU.add)

                # ---- bottom-half updates ----
                bt = slice(HWD, NW)
                nc.vector.tensor_tensor(i2[:, bt], dt0[:, bt], ps[:, bt],
                                        ALU.mult)
                nc.vector.tensor_tensor(flu[:, bt], flu[:, bt], i2[:, bt],
                                        ALU.add)
                nc.vector.tensor_tensor(g2[:, bt], dt0[:, bt], g2[:, bt],
                                        ALU.mult)
                nc.gpsimd.tensor_tensor(flv2[:, bt], flv2[:, bt], g2[:, bt],
                                        ALU.add)
                nc.sync.dma_start(
                    OUT[b, 256:512].rearrange("(rb p) w c -> p rb w c",
                                              p=128),
                    fl[:, bt, :].rearrange("p (rb w) c -> p rb w c", rb=2),
                )
    if legalize:
        legalize_single_wait(nc)
    return nc


# ---------------------------------------------------------------------------
# Post-pass: this walrus build encodes a single sync-wait slot per TPB
# instruction. Tile's sem assignment can emit 2+ waits on one instruction;
# hoist all but the last wait onto same-engine EventSemaphore carriers placed
# immediately before it (the sequencer then waits sequentially, which is
# semantically identical).
def legalize_single_wait(nc):
    import bass_rust

    capped = {
        mybir.EngineType.Activation,
        mybir.EngineType.DVE,
        mybir.EngineType.Pool,
        mybir.EngineType.PE,
        mybir.EngineType.SP,
    }
    exempt = {"EventSemaphore", "NoOp", "TriggerDma"}
    n = 0
    for fn in nc.m.functions:
        for blk in fn.blocks:
            insts = blk.instructions  # live list
            rebuilt = []
            changed = False
            for inst in list(insts):
                si = inst.sync_info
                waits = list(si.on_wait) if si is not None else []
                if (
                    len(waits) > 1
                    and inst.engine in capped
                    and str(inst.opcode) not in exempt
                ):
                    for w in waits[:-1]:
                        ev = mybir.InstEventSemaphore(
                            name=f"waitcarrier_{inst.name}_{n}", ins=[], outs=[]
                        )
                        ev.engine = inst.engine
                        ev.sync_info = bass_rust.SyncInfo(
                            on_wait=[w], on_update=[]
                        )
                        rebuilt.append(ev)
                        n += 1
                    inst.sync_info = bass_rust.SyncInfo(
                        on_wait=[waits[-1]], on_update=list(si.on_update)
                    )
                    changed = True
                rebuilt.append(inst)
            if changed:
                insts[:] = rebuilt
    return n


def _img_consts(P3: np.ndarray) -> np.ndarray:
    """9 warp consts F[i,j] (row-major) for one image's 3x3 corner P3[y,x].

    warped = sum_ij F'[i,j]*ay_i*ax_j, ax=(1,t2x,relu(t2x-1022)),
    ay=(1,t2y,relu(t2y-1022));  F = -0.1*F'.
    """
    P = P3.astype(np.float64)
    E = np.stack([P[:, 0], P[:, 1] - P[:, 0], P[:, 2] - P[:, 1]], axis=1)
    D = np.stack([E[0], E[1] - E[0], E[2] - E[1]], axis=0)
    r = 1.0 / 511.0
    Mx = np.array([[1.0, 0.0, 0.0], [-1.0, r, -r], [0.0, 0.0, r]])
    F = -0.1 * (Mx.T @ D @ Mx)
    return F.reshape(-1).astype(np.float32)


def host_consts(I1c: np.ndarray, n_rb: int = 4, hz: int = 253) -> np.ndarray:
    """Per-image folded warp coefficients + per-partition 2*h columns.

    I1c: [n_imgs, H, W] float32.  Returns [128, 9*n_imgs + n_rb + 10] f32.
    Per image b, cols 9*b+3*i+j hold F[i,j].  Col 9n+rb: 2*(128*rb+p).
    Col 9n+n_rb: band 2*h.  Cols 9n+n_rb+1..+9: band-partition-layout
    consts (partition NBR*b+r holds image b's values).
    """
    f = np.float32
    n_imgs = I1c.shape[0]
    cc = np.zeros((128, 9 * n_imgs + n_rb + 10), dtype=np.float32)
    allc = np.zeros((n_imgs, 9), dtype=np.float32)
    for b in range(n_imgs):
        allc[b] = _img_consts(I1c[b, 0:3, 0:3])
        cc[:, 9 * b : 9 * b + 9] = allc[b][None, :]
    p = np.arange(128, dtype=np.float32)
    for rb in range(n_rb):
        cc[:, 9 * n_imgs + rb] = f(2.0) * (f(128.0 * rb) + p)
    # band columns (NBR = 257-hz rows per image)
    base = 9 * n_imgs + n_rb
    nbr = 257 - hz
    for b in range(n_imgs):
        for r in range(nbr):
            pp = nbr * b + r
            if pp < 128:
                cc[pp, base] = f(2.0) * f(hz + r)
                cc[pp, base + 1 : base + 10] = allc[b]
    return cc


def host_gx() -> np.ndarray:
    w2 = (np.float32(2.0) * np.arange(512, dtype=np.float32)).astype(np.float32)
    return np.tile(w2, (128, 2)).astype(np.float32)


def host_sm() -> np.ndarray:
    """[128, 384] bf16: cols 0:128 = shift lhsT S (S[k,m]: +1 at k=m+1,
    -1 at k=m), cols 128:256 = patch lhsT (+1 at k=0, m=127), cols
    256:384 = S with column 127 zeroed (dy row 511 must be exactly 0)."""
    sm = np.zeros((128, 384), dtype=np.float32)
    for m in range(128):
        sm[m, m] = -1.0
        if m + 1 < 128:
            sm[m + 1, m] = 1.0
    sm[0, 128 + 127] = 1.0
    sm[:, 256:384] = sm[:, 0:128]
    sm[127, 256 + 127] = 0.0
    return sm.astype(ml_dtypes.bfloat16)


_NC = None
_NC_KEY = None


def _get_nc(wz, hz, wze, hze):
    global _NC, _NC_KEY
    if _NC is None or _NC_KEY != (wz, hz, wze, hze):
        _NC = build_nc(4, 4, wz=wz, hz=hz, wze=wze, hze=hze)
        _NC_KEY = (wz, hz, wze, hze)
    return _NC


def _splits(flow):
    umax = float(max(flow[..., 0].max(), 0.0))
    vmax = float(max(flow[..., 1].max(), 0.0))
    # first col/row where 2*x + d can reach 511.0 (f32-exact threshold)
    wz = int(min(256, max(1, (511.0 - umax) // 2 + 1)))
    hz = int(min(256, max(225, (511.0 - vmax) // 2 + 1)))
    assert np.float32(2.0 * (wz - 1)) + np.float32(umax) < np.float32(511.0)
    assert np.float32(2.0 * (hz - 1)) + np.float32(vmax) < np.float32(511.0)
    # first col/row where 2*x + d can reach 1022.0 (EX/EY strips)
    wze = int(min(512, max(wz + 1, (1022.0 - umax) // 2 + 1)))
    hze = int(min(512, max(481, (1022.0 - vmax) // 2 + 1)))
    assert wze == 512 or (
        np.float32(2.0 * (wze - 1)) + np.float32(umax) < np.float32(1022.0)
    )
    assert hze == 512 or (
        np.float32(2.0 * (hze - 1)) + np.float32(vmax) < np.float32(1022.0)
    )
    return wz, hz, wze, hze


def _make_in_maps(I1, I2, flow, wz, hz, n_cores=8):
    per = I1.shape[0] // n_cores
    gx = host_gx()
    sm = host_sm()
    in_maps = []
    for c in range(n_cores):
        sl = slice(c * per, (c + 1) * per)
        i1c = np.ascontiguousarray(I1[sl, :, :, 0], dtype=np.float32)
        in_maps.append(
            {
                "I1": i1c,
                "I2": np.ascontiguousarray(I2[sl, :, :, 0], dtype=np.float32),
                "FL": np.ascontiguousarray(flow[sl], dtype=np.float32),
                "CC": host_consts(i1c, 4, hz),
                "GX": gx,
                "SM": sm,
            }
        )
    return in_maps


def run(I1, I2, flow, trace=False, **kw):
    wz, hz, wze, hze = _splits(np.asarray(flow))
    nc = _get_nc(wz, hz, wze, hze)
    in_maps = _make_in_maps(I1, I2, flow, wz, hz)
    res = run_bass_kernel_spmd(nc, in_maps, list(range(8)), trace=trace, **kw)
    out = np.concatenate([r["OUT"] for r in res.results], axis=0)
    return out, res


def kernel(I1, I2, flow):
    out, _ = run(I1, I2, flow)
    return out.astype(np.float32)
